# revision 1
# baseline (speedup 1.0000x reference)
"""GPT2 eager causal attention (B=2, S=2048, D=1024, H=16, HD=64) on 8 TRN2 NeuronCores.

Sharding (data + head/tensor parallel, per the problem's hint):
  core c -> (batch b = c//4, head-group g = c%4) -- 4 heads per group.

Per-core pipeline (all layouts chosen so no score-matrix transpose is ever needed):
  1. x[b] transposed on PE -> xT [d, s]                      (d on partitions)
  2. QT,KT = wq/wk^T @ xT  -> [256, S] transposed layouts    (head-dim on partitions)
     V     = xT^T @ wv_ext -> [S, 260] natural, with a ones-column per head
  3. scores^T tiles ST[k, q] = KT_h^T-slices @ QT_h-slices   (k on partitions)
     exp on ScalarE with the 1/sqrt(64) scale folded in; causal masking via
     precomputed mask tiles on diagonal blocks only
     OT[d, q] += V^T-slices @ ST_exp : the ones-column makes row 64 the softmax
     denominator for free; normalize OT by its reciprocal (gpsimd broadcast)
  4. c_proj partial = OT^T-slices @ w_proj[group rows]
  5. ReduceScatter(add) over each 4-core (same-batch) group; each core emits its
     [512, 1024] token slice; host reassembles the [2, 2048, 1024] output.

Matmuls run as float32r (full-rate on PE for free dim >= 256, ~tf32 precision),
fp32 accumulation in PSUM, all storage fp32.
"""
from contextlib import ExitStack

import ml_dtypes
import numpy as np

import concourse.bacc as bacc
import concourse.mybir as mybir
import concourse.tile as tile
from concourse.bass_utils import run_bass_kernel_spmd

F32 = mybir.dt.float32
F32R = mybir.dt.float32r
BF16 = mybir.dt.bfloat16

B, S, D, H, HD = 2, 2048, 1024, 16, 64
N_CORES = 8
HG = 4               # heads per group
DG = HG * HD         # 256 q/k channels per group
VW = HG * (HD + 1)   # 260: 64 v-cols + 1 ones-col per head
NK = D // 128        # 8 contraction tiles over d
NS = S // 128        # 16 token tiles
CH = 512             # q-chunk (one PSUM bank of fp32)
NCH = S // CH        # 4
NRT = DG // 128      # 2 channel row-tiles per group


def _build(has_bv: bool, has_bp: bool, has_bqk: bool = False, tail: str = "rs", phases: int = 99):
    nc = bacc.Bacc("TRN2", target_bir_lowering=False, debug=False, num_devices=N_CORES)

    x_d = nc.dram_tensor("x", [S, D], BF16, kind="ExternalInput").ap()
    wq_d = nc.dram_tensor("wq", [D, DG], BF16, kind="ExternalInput").ap()
    wk_d = nc.dram_tensor("wk", [D, DG], BF16, kind="ExternalInput").ap()
    wv_d = nc.dram_tensor("wv", [D, VW], BF16, kind="ExternalInput").ap()
    wp_d = nc.dram_tensor("wp", [DG, D], BF16, kind="ExternalInput").ap()
    bq_d = nc.dram_tensor("bq", [DG, 1], F32, kind="ExternalInput").ap()
    bk_d = nc.dram_tensor("bk", [DG, 1], F32, kind="ExternalInput").ap()
    bv_d = nc.dram_tensor("bv", [DG, 1], F32, kind="ExternalInput").ap()
    bp_d = nc.dram_tensor("bp", [128, D], F32, kind="ExternalInput").ap()
    mk_d = nc.dram_tensor("masks", [128, 128], BF16, kind="ExternalInput").ap()
    if tail == "rs":
        out_d = nc.dram_tensor("out", [CH, D], F32, kind="ExternalOutput").ap()
    else:  # debug: emit the full per-core partial
        out_d = nc.dram_tensor("out", [S, D], F32, kind="ExternalOutput").ap()

    EXP = mybir.ActivationFunctionType.Exp
    IDENT = mybir.ActivationFunctionType.Identity

    with ExitStack() as ctx:
        tc = ctx.enter_context(tile.TileContext(nc))
        wpool = ctx.enter_context(tc.tile_pool(name="w", bufs=1))
        big = ctx.enter_context(tc.tile_pool(name="big", bufs=8))
        qkvp = ctx.enter_context(tc.tile_pool(name="qkv", bufs=1))
        stp = ctx.enter_context(tc.tile_pool(name="stx", bufs=6))
        nrm = ctx.enter_context(tc.tile_pool(name="nrm", bufs=2))
        outp = ctx.enter_context(tc.tile_pool(name="outp", bufs=3))
        ps_mm = ctx.enter_context(tc.tile_pool(name="psmm", bufs=3, space="PSUM"))
        ps_st = ctx.enter_context(tc.tile_pool(name="psst", bufs=3, space="PSUM"))
        ps_ot = ctx.enter_context(tc.tile_pool(name="psot", bufs=2, space="PSUM"))
        dram = ctx.enter_context(tc.tile_pool(name="dram", bufs=1, space="DRAM"))

        # ---- constants / weights -> SBUF
        wq_sb = wpool.tile([128, NK * DG], BF16)
        wk_sb = wpool.tile([128, NK * DG], BF16)
        wv_sb = wpool.tile([128, NK * VW], BF16)
        wp_sb = wpool.tile([128, NRT * D], BF16)
        mk_sb = wpool.tile([128, 128], BF16)
        on_sb = wpool.tile([1, 64], F32)
        bq_sb = wpool.tile([128, NRT], F32)
        bk_sb = wpool.tile([128, NRT], F32)
        bv_sb = wpool.tile([128, NRT], F32) if has_bv else None
        bp_sb = wpool.tile([128, D], F32) if has_bp else None
        for kt in range(NK):
            nc.sync.dma_start(wq_sb[:, kt * DG:(kt + 1) * DG], wq_d[kt * 128:(kt + 1) * 128, :])
            nc.sync.dma_start(wk_sb[:, kt * DG:(kt + 1) * DG], wk_d[kt * 128:(kt + 1) * 128, :])
            nc.sync.dma_start(wv_sb[:, kt * VW:(kt + 1) * VW], wv_d[kt * 128:(kt + 1) * 128, :])
        for rt in range(NRT):
            nc.sync.dma_start(wp_sb[:, rt * D:(rt + 1) * D], wp_d[rt * 128:(rt + 1) * 128, :])
            nc.sync.dma_start(bq_sb[:, rt:rt + 1], bq_d[rt * 128:(rt + 1) * 128, :])
            nc.sync.dma_start(bk_sb[:, rt:rt + 1], bk_d[rt * 128:(rt + 1) * 128, :])
            if has_bv:
                nc.sync.dma_start(bv_sb[:, rt:rt + 1], bv_d[rt * 128:(rt + 1) * 128, :])
        if has_bp:
            nc.sync.dma_start(bp_sb[:], bp_d[:])
        nc.sync.dma_start(mk_sb[:], mk_d[:])
        nc.vector.memset(on_sb[:], 1.0)

        # ---- phase 1: xT strips [128 d, S] via transpose-DMA (bf16 xbar path),
        # split into column chunks; weights were enqueued first so QKV can
        # start as soon as the sq=0 chunks land
        xT = []
        for dt in range(NK):
            t = big.tile([128, S], BF16, tag="bigslot", name=f"xT{dt}")
            xT.append(t)
        for sq in range(4):
            for dt in range(NK):
                nc.sync.dma_start_transpose(
                    xT[dt][:, sq * CH:(sq + 1) * CH],
                    x_d[sq * CH:(sq + 1) * CH, dt * 128:(dt + 1) * 128],
                )


        # ---- phase 2: QT/KT [256, S] (as 2 tiles of [128, S]) and V strips
        QT, KT = [], []
        for store, w_sb, b_sb, nm in ((QT, wq_sb, bq_sb, "q"), (KT, wk_sb, bk_sb, "k")):
            for rt in range(NRT):
                dst = qkvp.tile([128, S], BF16, tag=f"{nm}t{rt}", name=f"{nm}T{rt}")
                store.append(dst)
                for ch in range(NCH):
                    ps = ps_mm.tile([128, CH], F32, tag="ps", name=f"ps{nm}{rt}_{ch}")
                    for kt in range(NK):
                        nc.tensor.matmul(
                            ps[:],
                            (w_sb[:, kt * DG + rt * 128: kt * DG + (rt + 1) * 128]),
                            (xT[kt][:, ch * CH:(ch + 1) * CH]),
                            start=(kt == 0), stop=(kt == NK - 1),
                        )
                    if has_bqk:
                        nc.scalar.activation(
                            dst[:, ch * CH:(ch + 1) * CH], ps[:], IDENT,
                            bias=b_sb[:, rt:rt + 1],
                        )
                    else:
                        nc.vector.tensor_copy(dst[:, ch * CH:(ch + 1) * CH], ps[:])
        V = []
        for st in range(NS):
            vt = qkvp.tile([128, VW], BF16, tag=f"v{st}", name=f"v{st}")
            ps = ps_mm.tile([128, CH], F32, tag="ps", name=f"psv{st}")
            for kt in range(NK):
                nc.tensor.matmul(
                    ps[:, :VW],
                    (xT[kt][:, st * 128:(st + 1) * 128]),
                    (wv_sb[:, kt * VW:(kt + 1) * VW]),
                    start=(kt == 0), stop=(kt == NK - 1),
                )
            nc.vector.tensor_copy(vt[:], ps[:, :VW])
            for hl in range(HG):
                ones_col = vt[:, hl * (HD + 1) + HD: (hl + 1) * (HD + 1)].bitcast(mybir.dt.uint16)
                nc.vector.memset(ones_col, 0x3F80)  # bits of bf16 1.0
            V.append(vt)

        # ---- phase 3: attention + c_proj, chunk by chunk
        OT = []
        for i in range(NRT):
            t = big.tile([128, S], BF16, tag="bigslot", name=f"OT{i}")
            OT.append(t)
        partials = []
        for ch in range(NCH):
            pt = dram.tile([CH, D], F32, tag=f"partial{ch}", name=f"partial{ch}")
            partials.append(pt)
        rs_outs = []

        def emit_rs(ch):
            # rank r of the quad receives tokens [512*ch + 128*r, +128)
            rs_c = dram.tile([128, D], F32, tag=f"rs{ch}", name=f"rs_out{ch}")
            nc.gpsimd.collective_compute(
                "ReduceScatter",
                mybir.AluOpType.add,
                replica_groups=[[0, 1, 2, 3], [4, 5, 6, 7]],
                ins=[partials[ch].opt()],
                outs=[rs_c.opt()],
            )
            rs_outs.append((ch, rs_c))
        for ch in range(NCH):
            nkt = 4 * (ch + 1)
            for hl in range(HG):
                qt = QT[hl // 2]
                ktile = KT[hl // 2]
                off = 64 * (hl % 2)
                ot_ps = ps_ot.tile([65, CH], F32, tag="ot", name=f"ot{ch}_{hl}")
                for kt in range(nkt):
                    st_ps = ps_st.tile([128, CH], F32, tag="st", name=f"st{ch}_{hl}_{kt}")
                    nc.tensor.matmul(
                        st_ps[:],
                        (ktile[off:off + 64, kt * 128:(kt + 1) * 128]),
                        (qt[off:off + 64, ch * CH:(ch + 1) * CH]),
                        start=True, stop=True,
                    )
                    st_sb = stp.tile([128, CH], BF16, tag="stsb", name=f"se{ch}_{hl}_{kt}")
                    d = kt - 4 * ch
                    if d < 0:
                        nc.scalar.activation(st_sb[:], st_ps[:], EXP, scale=0.125)
                    else:
                        # diagonal strip: exp only the valid suffix, zero the
                        # prefix, triangular-mask the 128-wide diagonal block
                        if d > 0:
                            zc = st_sb[:, 0:d * 128].bitcast(mybir.dt.uint16)
                            nc.vector.memset(zc, 0)
                        nc.scalar.activation(st_sb[:, d * 128:], st_ps[:, d * 128:], EXP, scale=0.125)
                        nc.vector.tensor_mul(
                            st_sb[:, d * 128:(d + 1) * 128],
                            st_sb[:, d * 128:(d + 1) * 128],
                            mk_sb[:, 0:128],
                        )
                    nc.tensor.matmul(
                        ot_ps[:],
                        (V[kt][:, hl * (HD + 1):(hl + 1) * (HD + 1)]),
                        (st_sb[:]),
                        start=(kt == 0), stop=(kt == nkt - 1),
                    )
                den = nrm.tile([1, CH], F32, tag="den", name=f"den{ch}_{hl}")
                nc.vector.tensor_copy(den[:], ot_ps[64:65, :])
                rden = nrm.tile([1, CH], F32, tag="rden", name=f"rden{ch}_{hl}")
                nc.vector.reciprocal_approx_fast(rden[:], den[:])
                # rank-1 PE matmul broadcasts the reciprocal row to 64
                # partitions (keeps gpsimd free for the collectives)
                rbc_ps = ps_mm.tile([64, CH], F32, tag="ps", name=f"rbc{ch}_{hl}")
                nc.tensor.matmul(rbc_ps[:], on_sb[:], rden[:], start=True, stop=True)
                ot_sb = nrm.tile([64, CH], BF16, tag="otsb", name=f"otsb{ch}_{hl}")
                nc.vector.tensor_copy(ot_sb[:], ot_ps[0:64, :])
                dst = OT[hl // 2][off:off + 64, ch * CH:(ch + 1) * CH]
                nc.vector.tensor_mul(dst, ot_sb[:], rbc_ps[:])
                if has_bv:
                    nc.vector.tensor_scalar_add(dst, dst, bv_sb[off:off + 64, hl // 2: hl // 2 + 1])
            # c_proj for this chunk's tokens
            for stl in range(4):
                tok = ch * CH + stl * 128
                for n in range(NRT):
                    po = ps_mm.tile([128, CH], F32, tag="ps", name=f"po{ch}_{stl}_{n}")
                    for k2 in range(NRT):
                        nc.tensor.matmul(
                            po[:],
                            (OT[k2][:, tok:tok + 128]),
                            (wp_sb[:, k2 * D + n * CH: k2 * D + (n + 1) * CH]),
                            start=(k2 == 0), stop=(k2 == NRT - 1),
                        )
                    ob = outp.tile([128, CH], F32, tag="ob", name=f"ob{ch}_{stl}_{n}")
                    if has_bp:
                        nc.vector.tensor_add(ob[:], po[:], bp_sb[:, n * CH:(n + 1) * CH])
                    else:
                        nc.vector.tensor_copy(ob[:], po[:])
                    if tail == "rs":
                        nc.sync.dma_start(partials[ch][stl * 128:(stl + 1) * 128, n * CH:(n + 1) * CH], ob[:])
                    else:
                        nc.sync.dma_start(out_d[tok:tok + 128, n * CH:(n + 1) * CH], ob[:])
            if tail == "rs":
                emit_rs(ch)
        if tail == "rs":
            # final out DMAs last: keeps the in-order sync queue from blocking
            # mid-kernel partial writes behind collective completion waits
            for ch, rs_c in rs_outs:
                nc.sync.dma_start(out_d[ch * 128:(ch + 1) * 128, :], rs_c[:])

    nc.compile()
    return nc


_prog_cache = {}


def _get_prog(has_bv, has_bp, has_bqk):
    key = (has_bv, has_bp, has_bqk)
    if key not in _prog_cache:
        _prog_cache[key] = _build(has_bv, has_bp, has_bqk)
    return _prog_cache[key]


def _prepare(x, w_attn, b_attn, w_proj, b_proj):
    x = np.asarray(x, dtype=np.float32)
    w_attn = np.asarray(w_attn, dtype=np.float32)
    b_attn = np.asarray(b_attn, dtype=np.float32)
    w_proj = np.asarray(w_proj, dtype=np.float32)
    b_proj = np.asarray(b_proj, dtype=np.float32)

    has_bv = bool(np.any(b_attn[2 * D:]))
    has_bp = bool(np.any(b_proj))
    has_bqk = bool(np.any(b_attn[:2 * D]))
    nc = _get_prog(has_bv, has_bp, has_bqk)

    ii = np.arange(128)[:, None]
    jj = np.arange(128)[None, :]
    masks = (jj >= ii).astype(np.float32).astype(ml_dtypes.bfloat16)

    in_maps = []
    for c in range(N_CORES):
        b, g = divmod(c, 4)
        q0 = g * DG
        k0 = D + g * DG
        v0 = 2 * D + g * DG
        wv_ext = np.zeros((D, VW), dtype=np.float32)
        for hl in range(HG):
            wv_ext[:, hl * (HD + 1):hl * (HD + 1) + HD] = w_attn[:, v0 + hl * HD: v0 + (hl + 1) * HD]
        if g == 0:
            bp_tile = np.broadcast_to(b_proj, (128, D)).astype(np.float32)
        else:
            bp_tile = np.zeros((128, D), dtype=np.float32)
        in_maps.append({
            "x": np.ascontiguousarray(x[b]).astype(ml_dtypes.bfloat16),
            "wq": np.ascontiguousarray(w_attn[:, q0:q0 + DG]).astype(ml_dtypes.bfloat16),
            "wk": np.ascontiguousarray(w_attn[:, k0:k0 + DG]).astype(ml_dtypes.bfloat16),
            "wv": wv_ext.astype(ml_dtypes.bfloat16),
            "wp": np.ascontiguousarray(w_proj[g * DG:(g + 1) * DG, :]).astype(ml_dtypes.bfloat16),
            "bq": np.ascontiguousarray(b_attn[q0:q0 + DG, None]),
            "bk": np.ascontiguousarray(b_attn[k0:k0 + DG, None]),
            "bv": np.ascontiguousarray(b_attn[v0:v0 + DG, None]),
            "bp": bp_tile,
            "masks": masks,
        })
    return nc, in_maps


def _assemble(results):
    out = np.empty((B, S, D), dtype=np.float32)
    for c in range(N_CORES):
        b, g = divmod(c, 4)
        o = results[c]["out"]
        for ch in range(NCH):
            tok = ch * CH + g * 128
            out[b, tok:tok + 128, :] = o[ch * 128:(ch + 1) * 128, :]
    return out


def kernel(x, w_attn, b_attn, w_proj, b_proj):
    nc, in_maps = _prepare(x, w_attn, b_attn, w_proj, b_proj)
    res = run_bass_kernel_spmd(nc, in_maps, list(range(N_CORES)))
    return _assemble(res.results)



# revision 13
# speedup vs baseline: 1.5585x; 1.5585x over previous
"""GPT2 eager causal attention (B=2, S=2048, D=1024, H=16, HD=64) on 8 TRN2 NeuronCores.

Sharding (data + head/tensor parallel): core c -> (batch b = c//4, head-group
g = c%4), 4 heads per group.

Per-core pipeline:
  1. host feeds x[b] pre-transposed -> xT [d, s] strips land via plain DMA
  2. QT,KT = wq/wk^T @ xT -> [256, S] transposed layouts (head-dim on partitions)
     V = xT^T @ wv -> [S, 4x65] natural with a ones-column per head (memset once)
  3. per q-chunk, head-PAIR loop: two score MMs at base partitions 0/64 pack the
     PE via row tiling (K=64 each); exp on ScalarE batched over the pair
     [128, 2, 512] with the 1/sqrt(64) scale folded in; causal diagonal blocks
     masked by one strided multiply; OT[65, q] += V^T @ exp(ST) per head -- the
     ones-column makes row 64 the softmax denominator; normalization via a K=2
     block-diag ones matmul broadcasting both heads' reciprocal rows at once,
     fused mul PSUM x PSUM -> SBUF bf16
  4. c_proj partial = OT^T-slices @ w_proj[group rows], bf16 partials
  5. bf16 ReduceScatter(add) per chunk over each 4-core group (last chunk split
     in two for a shorter tail); bf16 outputs, host converts/reassembles.

Emission interleaves QKV(ch+1) and c_proj(ch-1) groups into attention(ch) so the
PE stream stays dense (HAM stays at K=8/8) while ScalarE works through the exps.
"""
from contextlib import ExitStack

import ml_dtypes
import numpy as np

import concourse.bacc as bacc
import concourse.mybir as mybir
import concourse.tile as tile
from concourse.bass_utils import run_bass_kernel_spmd

F32 = mybir.dt.float32
BF16 = mybir.dt.bfloat16

B, S, D, H, HD = 2, 2048, 1024, 16, 64
N_CORES = 8
HG = 4               # heads per group
DG = HG * HD         # 256 q/k channels per group
NK = D // 128        # 8 contraction tiles over d
NS = S // 128        # 16 token tiles
CH = 512             # q-chunk (one PSUM bank of fp32)
NCH = S // CH        # 4
NRT = DG // 128      # 2 channel row-tiles per group


def _build(has_bv: bool, has_bp: bool, has_bqk: bool = False, dbg: bool = False):
    nc = bacc.Bacc("TRN2", target_bir_lowering=False, debug=False, num_devices=N_CORES)

    x_d = nc.dram_tensor("x", [D, S], BF16, kind="ExternalInput").ap()
    wq_d = nc.dram_tensor("wq", [128, NK * DG], BF16, kind="ExternalInput").ap()
    wk_d = nc.dram_tensor("wk", [128, NK * DG], BF16, kind="ExternalInput").ap()
    wv_d = nc.dram_tensor("wv", [128, NK * DG], BF16, kind="ExternalInput").ap()
    wp_d = nc.dram_tensor("wp", [128, NRT * D], BF16, kind="ExternalInput").ap()
    bq_d = nc.dram_tensor("bq", [128, NRT], F32, kind="ExternalInput").ap()
    bk_d = nc.dram_tensor("bk", [128, NRT], F32, kind="ExternalInput").ap()
    bv_d = nc.dram_tensor("bv", [128, NRT], F32, kind="ExternalInput").ap()
    bp_d = nc.dram_tensor("bp", [128, D], F32, kind="ExternalInput").ap()
    mk_d = nc.dram_tensor("mk", [128, 128], BF16, kind="ExternalInput").ap()
    out_d = nc.dram_tensor("out", [CH, D], BF16, kind="ExternalOutput").ap()
    if dbg:
        dbg_qt = nc.dram_tensor("dbg_qt", [128, NRT, S], BF16, kind="ExternalOutput").ap()
        dbg_kt = nc.dram_tensor("dbg_kt", [128, NRT, S], BF16, kind="ExternalOutput").ap()
        dbg_v = nc.dram_tensor("dbg_v", [128, NS, HG, HD + 1], BF16, kind="ExternalOutput").ap()
        dbg_ot = nc.dram_tensor("dbg_ot", [128, NRT, S], BF16, kind="ExternalOutput").ap()
        dbg_den = nc.dram_tensor("dbg_den", [1, NCH * 2, 2, CH], F32, kind="ExternalOutput").ap()
        dbg_par = nc.dram_tensor("dbg_par", [CH, NCH, D], BF16, kind="ExternalOutput").ap()

    EXP = mybir.ActivationFunctionType.Exp
    IDENT = mybir.ActivationFunctionType.Identity

    with ExitStack() as ctx:
        tc = ctx.enter_context(tile.TileContext(nc))
        persist = ctx.enter_context(tc.tile_pool(name="persist", bufs=1))
        stp = ctx.enter_context(tc.tile_pool(name="stp", bufs=3))
        rdp = ctx.enter_context(tc.tile_pool(name="rdp", bufs=2))
        obp = ctx.enter_context(tc.tile_pool(name="obp", bufs=3))
        ps_sc = ctx.enter_context(tc.tile_pool(name="ps_sc", bufs=2, space="PSUM"))
        ps_ot = ctx.enter_context(tc.tile_pool(name="ps_ot", bufs=2, space="PSUM"))
        ps_ms = ctx.enter_context(tc.tile_pool(name="ps_ms", bufs=2, space="PSUM"))
        dram = ctx.enter_context(tc.tile_pool(name="dram", bufs=1, space="DRAM"))

        # ---- persistent SBUF tiles
        wq_sb = persist.tile([128, NK * DG], BF16)
        wk_sb = persist.tile([128, NK * DG], BF16)
        wv_sb = persist.tile([128, NK * DG], BF16)
        wp_sb = persist.tile([128, NRT * D], BF16)
        mk_sb = persist.tile([128, 2, 128], BF16)
        on1 = persist.tile([1, 64], BF16)
        bq_sb = persist.tile([128, NRT], F32) if has_bqk else None
        bk_sb = persist.tile([128, NRT], F32) if has_bqk else None
        bv_sb = persist.tile([128, NRT], F32) if has_bv else None
        bp_sb = persist.tile([128, D], F32) if has_bp else None
        xT = [persist.tile([128, S], BF16, name=f"xT{d}") for d in range(NK)]
        QT = [persist.tile([128, S], BF16, name=f"qT{r}") for r in range(NRT)]
        KT = [persist.tile([128, S], BF16, name=f"kT{r}") for r in range(NRT)]
        OTsb = [persist.tile([128, S], BF16, name=f"oT{r}") for r in range(NRT)]
        V_all = persist.tile([128, NS, HG, HD + 1], BF16)

        # ---- input DMAs (weights first so QKV can start as soon as x lands)
        nc.sync.dma_start(wq_sb[:], wq_d[:])
        nc.sync.dma_start(wk_sb[:], wk_d[:])
        nc.sync.dma_start(wv_sb[:], wv_d[:])
        nc.sync.dma_start(wp_sb[:], wp_d[:])
        for j in range(2):
            nc.sync.dma_start(mk_sb[:, j, :], mk_d[:])
        if has_bqk:
            nc.sync.dma_start(bq_sb[:], bq_d[:])
            nc.sync.dma_start(bk_sb[:], bk_d[:])
        if has_bv:
            nc.sync.dma_start(bv_sb[:], bv_d[:])
        if has_bp:
            nc.sync.dma_start(bp_sb[:], bp_d[:])
        for ch in range(NCH):
            for dt in range(NK):
                nc.sync.dma_start(
                    xT[dt][:, ch * CH:(ch + 1) * CH],
                    x_d[dt * 128:(dt + 1) * 128, ch * CH:(ch + 1) * CH],
                )

        nc.vector.memset(on1[:], 1.0)
        nc.vector.memset(V_all[:, :, :, HD:HD + 1], 1.0)

        # ---- PE warmup: keep the array busy through the HAM window while x lands
        for i in range(12):
            wps = ps_ms.tile([128, CH], F32, tag="mm", name=f"warm{i}")
            nc.tensor.matmul(wps[:], wq_sb[:, 0:128], wq_sb[:, 0:CH], start=True, stop=True)

        # ---- QKV + c_proj group emitters (each rotates one misc PSUM bank)
        def emit_qkt_group(dst, w_sb, b_sb, rt, ch):
            ps = ps_ms.tile([128, CH], F32, tag="mm", name=f"qk{rt}_{ch}")
            for kt in range(NK):
                nc.tensor.matmul(
                    ps[:],
                    w_sb[:, kt * DG + rt * 128: kt * DG + (rt + 1) * 128],
                    xT[kt][:, ch * CH:(ch + 1) * CH],
                    start=(kt == 0), stop=(kt == NK - 1),
                )
            if has_bqk:
                nc.scalar.activation(
                    dst[:, ch * CH:(ch + 1) * CH], ps[:], IDENT,
                    bias=b_sb[:, rt:rt + 1],
                )
            else:
                nc.vector.tensor_copy(dst[:, ch * CH:(ch + 1) * CH], ps[:])

        def emit_v_group(st):
            ps = ps_ms.tile([128, HG, HD], F32, tag="mm", name=f"v{st}")
            for kt in range(NK):
                nc.tensor.matmul(
                    ps[:, :, :],
                    xT[kt][:, st * 128:(st + 1) * 128],
                    wv_sb[:, kt * DG:(kt + 1) * DG],
                    start=(kt == 0), stop=(kt == NK - 1),
                )
            nc.vector.tensor_copy(V_all[:, st, :, 0:HD], ps[:, :, :])

        partials = [
            dram.tile([CH, D], BF16, tag=f"partial{c}", name=f"partial{c}")
            for c in range(NCH)
        ]

        def emit_cproj_group(ch, stl, n):
            tok = ch * CH + stl * 128
            ps = ps_ms.tile([128, CH], F32, tag="mm", name=f"po{ch}_{stl}_{n}")
            for k2 in range(NRT):
                nc.tensor.matmul(
                    ps[:],
                    OTsb[k2][:, tok:tok + 128],
                    wp_sb[:, k2 * D + n * CH: k2 * D + (n + 1) * CH],
                    start=(k2 == 0), stop=(k2 == NRT - 1),
                )
            ob = obp.tile([128, CH], BF16, tag="ob", name=f"ob{ch}_{stl}_{n}")
            if has_bp:
                nc.vector.tensor_add(ob[:], ps[:], bp_sb[:, n * CH:(n + 1) * CH])
            else:
                nc.vector.tensor_copy(ob[:], ps[:])
            nc.sync.dma_start(
                partials[ch][stl * 128:(stl + 1) * 128, n * CH:(n + 1) * CH], ob[:]
            )

        out_off = [0]

        def emit_rs(in_ap, rows, nm):
            rs_c = dram.tile([rows, D], BF16, tag=f"rs{nm}", name=f"rs_{nm}")
            nc.gpsimd.collective_compute(
                "ReduceScatter",
                mybir.AluOpType.add,
                replica_groups=[[0, 1, 2, 3], [4, 5, 6, 7]],
                ins=[in_ap.opt()],
                outs=[rs_c.opt()],
            )
            o = out_off[0]
            nc.scalar.dma_start(out_d[o:o + rows, :], rs_c[:])
            out_off[0] = o + rows

        # ---- attention emitters
        def attn_unit(ch, p, kt, nkt, ot_pair):
            d = kt - 4 * ch  # >=0 on the causal diagonal strip
            sc = ps_sc.tile([128, 2, CH], F32, tag="sc", name=f"sc{ch}_{p}_{kt}")
            for j in range(2):
                off = 64 * j
                nc.tensor.matmul(
                    sc[:, j, :],
                    KT[p][off:off + 64, kt * 128:(kt + 1) * 128],
                    QT[p][off:off + 64, ch * CH:(ch + 1) * CH],
                    start=True, stop=True,
                )
            st = stp.tile([128, 2, CH], BF16, tag="st", name=f"st{ch}_{p}_{kt}")
            a = max(d, 0) * 128
            nc.scalar.activation(st[:, :, a:], sc[:, :, a:], EXP, scale=0.125)
            if d >= 0:
                nc.vector.tensor_mul(
                    st[:, :, d * 128:(d + 1) * 128],
                    st[:, :, d * 128:(d + 1) * 128],
                    mk_sb[:, :, :],
                )
            for j in range(2):
                hl = 2 * p + j
                nc.tensor.matmul(
                    ot_pair[j][:, a:],
                    V_all[:, kt, hl, :],
                    st[:, j, a:],
                    start=(kt == 0), stop=(kt == nkt - 1),
                )

        def pair_norm(ch, p, ot_pair):
            den = rdp.tile([1, 2, CH], F32, tag="den", name=f"den{ch}_{p}")
            nc.vector.tensor_copy(den[:, 0, :], ot_pair[0][64:65, :])
            nc.vector.tensor_copy(den[:, 1, :], ot_pair[1][64:65, :])
            rdf = rdp.tile([1, 2, CH], F32, tag="rdf", name=f"rdf{ch}_{p}")
            nc.vector.reciprocal_approx_fast(rdf[:], den[:])
            if dbg:
                nc.sync.dma_start(dbg_den[:, 2 * ch + p, :, :], den[:])
            rdb = rdp.tile([1, 2, CH], BF16, tag="rdb", name=f"rdb{ch}_{p}")
            nc.vector.tensor_copy(rdb[:], rdf[:])
            for j in range(2):
                # rank-1 broadcast of the reciprocal row to 64 partitions; one
                # PSUM tile per head (a second col-tiled matmul into the same
                # bank at tile_position (0,64) silently writes nothing on HW)
                rbp = ps_ms.tile([64, CH], F32, tag="mm", name=f"rb{ch}_{p}_{j}")
                nc.tensor.matmul(rbp[:], on1[:], rdb[:, j, :], start=True, stop=True)
                rbs = rdp.tile([64, CH], BF16, tag=f"rbs{j}", name=f"rbs{ch}_{p}_{j}")
                nc.vector.tensor_copy(rbs[:], rbp[:])
                dst = OTsb[p][64 * j:64 * j + 64, ch * CH:(ch + 1) * CH]
                nc.vector.tensor_mul(dst, ot_pair[j][0:64, :], rbs[:])
                if has_bv:
                    nc.vector.tensor_scalar_add(dst, dst, bv_sb[64 * j:64 * j + 64, p:p + 1])

        def qkv_groups(ch):
            gs = []
            for rt in range(NRT):
                gs.append(lambda rt=rt, ch=ch: emit_qkt_group(QT[rt], wq_sb, bq_sb, rt, ch))
                gs.append(lambda rt=rt, ch=ch: emit_qkt_group(KT[rt], wk_sb, bk_sb, rt, ch))
            for st4 in range(4):
                gs.append(lambda st=4 * ch + st4: emit_v_group(st))
            return gs

        # ---- prologue: QKV for chunk 0
        for g in qkv_groups(0):
            g()

        # ---- chunk loop: attention(ch) with QKV(ch+1) + c_proj(ch-1) interleaved
        for ch in range(NCH):
            nkt = 4 * (ch + 1)
            fillers = []
            if ch > 0:
                for stl in range(4):
                    for n in range(NRT):
                        fillers.append(
                            lambda ch=ch - 1, stl=stl, n=n: emit_cproj_group(ch, stl, n)
                        )
                fillers.append(lambda ch=ch - 1: emit_rs(partials[ch][:, :], 128, f"c{ch}"))
            if ch + 1 < NCH:
                fillers += qkv_groups(ch + 1)

            n_units = 2 * nkt
            fi = 0
            ui = 0
            for p in range(2):
                ot_pair = [
                    ps_ot.tile([65, CH], F32, tag="ot", name=f"ot{ch}_{p}_{j}")
                    for j in range(2)
                ]
                for kt in range(nkt):
                    attn_unit(ch, p, kt, nkt, ot_pair)
                    ui += 1
                    want = (ui * len(fillers)) // n_units
                    while fi < want:
                        fillers[fi]()
                        fi += 1
                pair_norm(ch, p, ot_pair)
            while fi < len(fillers):
                fillers[fi]()
                fi += 1

        # ---- epilogue: c_proj(last) + split RS for a short tail
        for stl in range(4):
            for n in range(NRT):
                emit_cproj_group(NCH - 1, stl, n)
        emit_rs(partials[NCH - 1][0:256, :], 64, "c3a")
        emit_rs(partials[NCH - 1][256:512, :], 64, "c3b")
        if dbg:
            for rt in range(NRT):
                nc.sync.dma_start(dbg_qt[:, rt, :], QT[rt][:])
                nc.sync.dma_start(dbg_kt[:, rt, :], KT[rt][:])
                nc.sync.dma_start(dbg_ot[:, rt, :], OTsb[rt][:])
            nc.sync.dma_start(dbg_v[:], V_all[:])
            for ch in range(NCH):
                nc.sync.dma_start(dbg_par[:, ch, :], partials[ch][:, :])

    nc.compile()
    return nc


_prog_cache = {}


def _get_prog(has_bv, has_bp, has_bqk):
    key = (has_bv, has_bp, has_bqk)
    if key not in _prog_cache:
        _prog_cache[key] = _build(has_bv, has_bp, has_bqk)
    return _prog_cache[key]


def _prepare(x, w_attn, b_attn, w_proj, b_proj):
    x = np.asarray(x, dtype=np.float32)
    w_attn = np.asarray(w_attn, dtype=np.float32)
    b_attn = np.asarray(b_attn, dtype=np.float32)
    w_proj = np.asarray(w_proj, dtype=np.float32)
    b_proj = np.asarray(b_proj, dtype=np.float32)

    has_bv = bool(np.any(b_attn[2 * D:]))
    has_bp = bool(np.any(b_proj))
    has_bqk = bool(np.any(b_attn[:2 * D]))
    nc = _get_prog(has_bv, has_bp, has_bqk)

    ii = np.arange(128)[:, None]
    jj = np.arange(128)[None, :]
    mask = (jj >= ii).astype(np.float32).astype(ml_dtypes.bfloat16)

    def tile_cols(w, c0, width):
        t = np.empty((128, NK * width), np.float32)
        for kt in range(NK):
            t[:, kt * width:(kt + 1) * width] = w[kt * 128:(kt + 1) * 128, c0:c0 + width]
        return t.astype(ml_dtypes.bfloat16)

    xb = [np.ascontiguousarray(x[b].T).astype(ml_dtypes.bfloat16) for b in range(B)]

    per_group = []
    for g in range(HG):
        q0 = g * DG
        k0 = D + g * DG
        v0 = 2 * D + g * DG
        wp_t = np.empty((128, NRT * D), np.float32)
        for rt in range(NRT):
            wp_t[:, rt * D:(rt + 1) * D] = w_proj[g * DG + rt * 128: g * DG + (rt + 1) * 128, :]
        bt = {}
        for nm, c0 in (("bq", q0), ("bk", k0), ("bv", v0)):
            t = np.empty((128, NRT), np.float32)
            for rt in range(NRT):
                t[:, rt] = b_attn[c0 + rt * 128: c0 + (rt + 1) * 128]
            bt[nm] = t
        if g == 0:
            bp_tile = np.broadcast_to(b_proj, (128, D)).astype(np.float32)
        else:
            bp_tile = np.zeros((128, D), dtype=np.float32)
        per_group.append({
            "wq": tile_cols(w_attn, q0, DG),
            "wk": tile_cols(w_attn, k0, DG),
            "wv": tile_cols(w_attn, v0, DG),
            "wp": wp_t.astype(ml_dtypes.bfloat16),
            "bq": np.ascontiguousarray(bt["bq"]),
            "bk": np.ascontiguousarray(bt["bk"]),
            "bv": np.ascontiguousarray(bt["bv"]),
            "bp": np.ascontiguousarray(bp_tile),
            "mk": mask,
        })

    in_maps = []
    for c in range(N_CORES):
        b, g = divmod(c, 4)
        m = dict(per_group[g])
        m["x"] = xb[b]
        in_maps.append(m)
    return nc, in_maps


def _assemble(results):
    out = np.empty((B, S, D), dtype=np.float32)
    for c in range(N_CORES):
        b, g = divmod(c, 4)
        o = np.asarray(results[c]["out"], dtype=np.float32)
        for ch in range(3):
            tok = ch * CH + g * 128
            out[b, tok:tok + 128, :] = o[ch * 128:(ch + 1) * 128, :]
        out[b, 1536 + g * 64:1536 + g * 64 + 64, :] = o[384:448, :]
        out[b, 1792 + g * 64:1792 + g * 64 + 64, :] = o[448:512, :]
    return out


def kernel(x, w_attn, b_attn, w_proj, b_proj):
    nc, in_maps = _prepare(x, w_attn, b_attn, w_proj, b_proj)
    res = run_bass_kernel_spmd(nc, in_maps, list(range(N_CORES)))
    return _assemble(res.results)
